# revision 15
# baseline (speedup 1.0000x reference)
"""Trainium2 Bass kernel for masked bi-linear attention.

Computes, for full inputs
    k:    [B, KL, E] f32
    q:    [B, Q,  E] f32
    W:    [E, E]     f32
    mask: [B, Q, KL] i32 (0/1)
the reference
    qw    = q @ W                      [B, Q, E]
    s     = qw @ k^T                   [B, Q, KL]
    p     = softmax(s, axis=-1) * mask
    out   = p @ k                      [B, Q, E]

Sharding: data-parallel over B across 8 NeuronCores (2 batches/core),
W replicated. Each core runs the same Bass program on its B-slice.

Precision: qw and score matmuls in float32r (PE truncates fp32 reads to
fp22, full bf16 rate) - scores carry ~13 mantissa bits, enough for the
peaked softmax (std ~32). p and the PV matmul run in bf16: p in [0,1]
rounds to ~0.2%, k rounds to ~0.2%, contributing a few e-3 of output
error on top of the ~1.2e-3 from fp22 scores.

Per q-tile (128 rows) steady state:
  PE:   8 score matmuls x 4 psum banks (f32r), then 32 PV matmuls (bf16)
        for the tile deferred two slots earlier.
  DVE:  evacuates score banks psum->sbuf + per-bank max, combines maxes,
        reciprocal of z, mask multiply (bf16).
  ACT:  exp (sbuf f32 -> bf16) with sum accumulator, PV output scale.
  DMA:  mask stream (scalar queue), one XBAR-transpose of the bf16 p
        tile [128,2048] -> [128,16,128] chunked pT (sync queue), output
        store (gpsimd queue).
The PV for tile n runs after score matmuls of tile n+2, so softmax
(DVE/ACT) and the transpose DMA of tile n complete under the PE's score
work of tiles n+1/n+2.
"""

import numpy as np

import concourse.bacc as bacc
import concourse.mybir as mybir
import concourse.tile as tile
from concourse.bass_utils import run_bass_kernel_spmd
from concourse.masks import make_identity
from contextlib import ExitStack

dt = mybir.dt
AF = mybir.ActivationFunctionType
ALU = mybir.AluOpType
AX = mybir.AxisListType

P = 128

N_CORES = 8
B, Q_LEN, K_LEN, EMB = 16, 2048, 2048, 1024


def emit_attention(ctx, tc, k_ap, q_ap, w_ap, mask_ap, out_ap,
                   Bl, Q, KL, E, QB=256):
    nc = tc.nc
    f32, bf16, i32, f32r = dt.float32, dt.bfloat16, dt.int32, dt.float32r

    assert Q % QB == 0 and QB % P == 0 and KL % P == 0 and E % P == 0
    EC = E // P          # e (contraction for qw) chunks
    KC = KL // P         # k chunks
    FC = E // P          # f chunks (qw output tiles)
    nqb = Q // QB
    qt_per_b = QB // P
    KB = 512             # score psum block (1 bank)
    nkb = KL // KB
    GW = 4               # transposes batched per psum bank

    const = ctx.enter_context(tc.tile_pool(name="const", bufs=1))
    ident = const.tile([P, P], f32)
    make_identity(nc, ident[:])

    big = ctx.enter_context(tc.tile_pool(name="big", bufs=1))
    qio = ctx.enter_context(tc.tile_pool(name="qio", bufs=4))
    mio = ctx.enter_context(tc.tile_pool(name="mio", bufs=3))
    oio = ctx.enter_context(tc.tile_pool(name="oio", bufs=1))
    spp = ctx.enter_context(tc.tile_pool(name="spp", bufs=2))
    e16p = ctx.enter_context(tc.tile_pool(name="e16p", bufs=2))
    ptp = ctx.enter_context(tc.tile_pool(name="ptp", bufs=3))
    small = ctx.enter_context(tc.tile_pool(name="small", bufs=3))
    psum = ctx.enter_context(tc.tile_pool(name="psum", bufs=4, space="PSUM"))
    psum_t = ctx.enter_context(tc.tile_pool(name="psum_t", bufs=2, space="PSUM"))
    psum_o = ctx.enter_context(tc.tile_pool(name="psum_o", bufs=1, space="PSUM"))

    # ---- W: loaded once per core into f32r (bit-identical copy);
    # emission deferred until after the first q-block's DMAs are queued
    wH = big.tile([P, EC * E], f32r, tag="wH")

    def emit_w_load():
        for ec in range(EC):
            win = qio.tile([P, E], f32, tag="qin", name="win")
            nc.scalar.dma_start(win[:], w_ap[ec * P:(ec + 1) * P, :])
            nc.scalar.copy(wH[:, ec * E:(ec + 1) * E], win[:])

    # deferred PV state: (b, row0, pt3, rz)
    pending = []

    def emit_pv(st, knat):
        b, row0, pt3, rz = st
        po = psum_o.tile([P, 2 * KB], f32, tag="po", name="po")
        for kc in range(KC):
            for eh in range(2):
                nc.tensor.matmul(
                    po[:, eh * KB:(eh + 1) * KB], pt3[:, kc, :],
                    knat[:, kc * E + eh * KB: kc * E + (eh + 1) * KB],
                    start=(kc == 0), stop=(kc == KC - 1))
        ot = oio.tile([P, 2 * KB], f32, tag="ot", name="ot")
        nc.scalar.activation(ot[:], po[:], AF.Copy, scale=rz[:])
        nc.gpsimd.dma_start(out_ap[b, row0: row0 + P, :], ot[:])

    def emit_block_qT(b, qb):
        q0 = qb * QB
        qTh = big.tile([P, EC, QB], f32r, tag="qTh", name="qTh")
        for qt in range(qt_per_b):
            qin = qio.tile([P, E], f32, tag="qin", name="qin")
            nc.sync.dma_start(
                qin[:], q_ap[b, q0 + qt * P: q0 + (qt + 1) * P, :])
            for eg in range(EC // GW):
                pt = psum_t.tile([P, GW * P], f32, tag="tp", name="pt")
                for j in range(GW):
                    ec = eg * GW + j
                    nc.tensor.transpose(
                        pt[:, j * P:(j + 1) * P],
                        qin[:, ec * P:(ec + 1) * P], ident[:])
                ptv = pt[:].rearrange("p (g c) -> p g c", g=GW)
                nc.scalar.copy(
                    qTh[:, eg * GW:(eg + 1) * GW, qt * P:(qt + 1) * P], ptv)
        return qTh

    def emit_block_qw(qTh):
        qwTh = big.tile([P, FC * QB], f32r, tag="qwTh", name="qwTh")
        for fg in range(FC // 2):
            ps = psum.tile([P, 2 * QB], f32, tag="ps", name="ps")
            for fi in range(2):
                fc = fg * 2 + fi
                for ec in range(EC):
                    nc.tensor.matmul(
                        ps[:, fi * QB:(fi + 1) * QB],
                        wH[:, ec * E + fc * P: ec * E + (fc + 1) * P],
                        qTh[:, ec, :],
                        start=(ec == 0), stop=(ec == EC - 1))
            nc.scalar.copy(qwTh[:, fg * 2 * QB:(fg + 1) * 2 * QB], ps[:])
        return qwTh

    def emit_k_phase(b):
        knat = big.tile([P, KC * E], bf16, tag="knat", name="knat")
        kTh = big.tile([P, EC, KL], f32r, tag="kTh", name="kTh")
        for kc in range(KC):
            kin = qio.tile([P, E], f32, tag="qin", name="kin")
            nc.sync.dma_start(kin[:], k_ap[b, kc * P:(kc + 1) * P, :])
            # bf16 copy for the PV matmul rhs
            nc.vector.tensor_copy(knat[:, kc * E:(kc + 1) * E], kin[:])
            for eg in range(EC // GW):
                pt = psum_t.tile([P, GW * P], f32, tag="tp", name="pt")
                for j in range(GW):
                    ec = eg * GW + j
                    nc.tensor.transpose(
                        pt[:, j * P:(j + 1) * P],
                        kin[:, ec * P:(ec + 1) * P], ident[:])
                ptv = pt[:].rearrange("p (g c) -> p g c", g=GW)
                nc.scalar.copy(
                    kTh[:, eg * GW:(eg + 1) * GW, kc * P:(kc + 1) * P], ptv)
        return knat, kTh

    for b in range(Bl):
        # first q-block prep runs before the K phase so the PE has work
        # while the k DMA stream lands
        qTh = emit_block_qT(b, 0)
        if b == 0:
            emit_w_load()
        qwTh = emit_block_qw(qTh)
        # flush deferred PVs of the previous batch before knat is rewritten
        while pending:
            emit_pv(pending.pop(0), knat)
        knat, kTh = emit_k_phase(b)

        qTh_next = None
        for qb in range(nqb):
            q0 = qb * QB
            if qb > 0:
                # qTh for this block was transposed mid-previous-block;
                # only the qw matmuls run here (their psum use is
                # sequential with this block's score matmuls)
                qwTh = emit_block_qw(qTh_next)

            for qt in range(qt_per_b):
                if qt == 1 and qb + 1 < nqb:
                    # transpose the next block's q mid-block so its ACT
                    # psum evacuations finish under this block's
                    # remaining score/PV matmuls
                    qTh_next = emit_block_qT(b, qb + 1)
                sp = spp.tile([P, KL], f32, tag="sp", name="sp")
                m4 = small.tile([P, nkb], f32, tag="m4", name="m4")
                for kb in range(nkb):
                    ps_s = psum.tile([P, KB], f32, tag="ps", name="ps_s")
                    for fc in range(FC):
                        qs = fc * QB + qt * P
                        nc.tensor.matmul(ps_s[:], qwTh[:, qs:qs + P],
                                         kTh[:, fc, kb * KB:(kb + 1) * KB],
                                         start=(fc == 0),
                                         stop=(fc == FC - 1))
                    nc.vector.tensor_copy(sp[:, kb * KB:(kb + 1) * KB],
                                          ps_s[:])
                    nc.vector.tensor_reduce(m4[:, kb:kb + 1], ps_s[:],
                                            axis=AX.X, op=ALU.max)

                negm = small.tile([P, 1], f32, tag="negm", name="negm")
                nc.vector.tensor_reduce(negm[:], m4[:], axis=AX.X,
                                        op=ALU.max, negate=True)
                z = small.tile([P, 1], f32, tag="z", name="z")
                e16 = e16p.tile([P, KL], bf16, tag="e16", name="e16")
                nc.scalar.activation(e16[:], sp[:], AF.Exp,
                                     bias=negm[:], accum_out=z[:])
                rz = small.tile([P, 1], f32, tag="rz", name="rz")
                nc.vector.reciprocal(rz[:], z[:])

                # multiplicative mask (applied after softmax numerator),
                # in place on the bf16 exp tile
                for kb in range(nkb):
                    mt = mio.tile([P, KB], i32, tag="mask", name="mt")
                    # gpsimd (SWDGE) dispatch: keeps the mask stream off
                    # the ACT queue so slot-waits can never delay exp
                    nc.gpsimd.dma_start(
                        mt[:], mask_ap[b, q0 + qt * P: q0 + (qt + 1) * P,
                                       kb * KB:(kb + 1) * KB])
                    nc.vector.scalar_tensor_tensor(
                        out=e16[:, kb * KB:(kb + 1) * KB], in0=mt[:],
                        scalar=1.0, in1=e16[:, kb * KB:(kb + 1) * KB],
                        op0=ALU.mult, op1=ALU.mult)

                # one XBAR transpose: p [128q, KL] -> pT chunks [128l, KC, 128q]
                pt3 = ptp.tile([P, KC, P], bf16, tag="pt3", name="pt3")
                nc.scalar.dma_start(pt3[:], e16[:], transpose=True)

                pending.append((b, q0 + qt * P, pt3, rz))
                if len(pending) > 2:
                    emit_pv(pending.pop(0), knat)

    while pending:
        emit_pv(pending.pop(0), knat)


def build_program(Bl, Q, KL, E, QB=256):
    nc = bacc.Bacc("TRN2", target_bir_lowering=False, debug=False)
    k_t = nc.dram_tensor("k", [Bl, KL, E], dt.float32, kind="ExternalInput")
    q_t = nc.dram_tensor("q", [Bl, Q, E], dt.float32, kind="ExternalInput")
    w_t = nc.dram_tensor("W", [E, E], dt.float32, kind="ExternalInput")
    m_t = nc.dram_tensor("mask", [Bl, Q, KL], dt.int32, kind="ExternalInput")
    o_t = nc.dram_tensor("out", [Bl, Q, E], dt.float32, kind="ExternalOutput")
    with tile.TileContext(nc) as tc:
        with ExitStack() as ctx:
            emit_attention(ctx, tc, k_t.ap(), q_t.ap(), w_t.ap(), m_t.ap(),
                           o_t.ap(), Bl, Q, KL, E, QB=QB)
    nc.compile()
    return nc


def kernel(k: np.ndarray, q: np.ndarray, W: np.ndarray, mask: np.ndarray,
           **run_kwargs) -> np.ndarray:
    assert k.shape == (B, K_LEN, EMB) and q.shape == (B, Q_LEN, EMB)
    assert W.shape == (EMB, EMB) and mask.shape == (B, Q_LEN, K_LEN)
    Bl = B // N_CORES
    nc = build_program(Bl, Q_LEN, K_LEN, EMB)
    in_maps = []
    for c in range(N_CORES):
        sl = slice(c * Bl, (c + 1) * Bl)
        in_maps.append({
            "k": np.ascontiguousarray(k[sl], dtype=np.float32),
            "q": np.ascontiguousarray(q[sl], dtype=np.float32),
            "W": np.ascontiguousarray(W, dtype=np.float32),
            "mask": np.ascontiguousarray(mask[sl], dtype=np.int32),
        })
    res = run_bass_kernel_spmd(nc, in_maps, core_ids=list(range(N_CORES)),
                               **run_kwargs)
    out = np.concatenate([r["out"] for r in res.results], axis=0)
    if run_kwargs.get("trace"):
        kernel.last_exec_time_ns = res.exec_time_ns
    return out


kernel.last_exec_time_ns = None


# revision 16
# speedup vs baseline: 1.1550x; 1.1550x over previous
"""Trainium2 Bass kernel for masked bi-linear attention.

Computes, for full inputs
    k:    [B, KL, E] f32
    q:    [B, Q,  E] f32
    W:    [E, E]     f32
    mask: [B, Q, KL] i32 (0/1)
the reference
    qw    = q @ W                      [B, Q, E]
    s     = qw @ k^T                   [B, Q, KL]
    p     = softmax(s, axis=-1) * mask
    out   = p @ k                      [B, Q, E]

Sharding: data-parallel over B across 8 NeuronCores (2 batches/core),
W replicated. Each core runs the same Bass program on its B-slice.

Precision: qw and score matmuls in float32r (PE truncates fp32 reads to
fp22, full bf16 rate) - scores carry ~13 mantissa bits, enough for the
peaked softmax (std ~32). p and the PV matmul run in bf16: p in [0,1]
rounds to ~0.2%, k rounds to ~0.2%, contributing a few e-3 of output
error on top of the ~1.2e-3 from fp22 scores.

Per q-tile (128 rows) steady state:
  PE:   8 score matmuls x 4 psum banks (f32r), then 32 PV matmuls (bf16)
        for the tile deferred two slots earlier.
  DVE:  evacuates score banks psum->sbuf + per-bank max, combines maxes,
        reciprocal of z, mask multiply (bf16).
  ACT:  exp (sbuf f32 -> bf16) with sum accumulator, PV output scale.
  DMA:  mask stream (scalar queue), one XBAR-transpose of the bf16 p
        tile [128,2048] -> [128,16,128] chunked pT (sync queue), output
        store (gpsimd queue).
The PV for tile n runs after score matmuls of tile n+2, so softmax
(DVE/ACT) and the transpose DMA of tile n complete under the PE's score
work of tiles n+1/n+2.
"""

import numpy as np

import concourse.bacc as bacc
import concourse.mybir as mybir
import concourse.tile as tile
from concourse.bass_utils import run_bass_kernel_spmd
from concourse.masks import make_identity
from contextlib import ExitStack

dt = mybir.dt
AF = mybir.ActivationFunctionType
ALU = mybir.AluOpType
AX = mybir.AxisListType

P = 128

N_CORES = 8
B, Q_LEN, K_LEN, EMB = 16, 2048, 2048, 1024


def emit_attention(ctx, tc, k_ap, q_ap, w_ap, mask_ap, out_ap,
                   Bl, Q, KL, E, QB=256):
    nc = tc.nc
    f32, bf16, i32, f32r = dt.float32, dt.bfloat16, dt.int32, dt.float32r

    assert Q % QB == 0 and QB % P == 0 and KL % P == 0 and E % P == 0
    EC = E // P          # e (contraction for qw) chunks
    KC = KL // P         # k chunks
    FC = E // P          # f chunks (qw output tiles)
    nqb = Q // QB
    qt_per_b = QB // P
    KB = 512             # score psum block (1 bank)
    nkb = KL // KB
    GW = 4               # transposes batched per psum bank

    const = ctx.enter_context(tc.tile_pool(name="const", bufs=1))
    ident = const.tile([P, P], f32)
    make_identity(nc, ident[:])

    big = ctx.enter_context(tc.tile_pool(name="big", bufs=1))
    qio = ctx.enter_context(tc.tile_pool(name="qio", bufs=4))
    mio = ctx.enter_context(tc.tile_pool(name="mio", bufs=3))
    oio = ctx.enter_context(tc.tile_pool(name="oio", bufs=1))
    spp = ctx.enter_context(tc.tile_pool(name="spp", bufs=2))
    e16p = ctx.enter_context(tc.tile_pool(name="e16p", bufs=2))
    ptp = ctx.enter_context(tc.tile_pool(name="ptp", bufs=3))
    small = ctx.enter_context(tc.tile_pool(name="small", bufs=3))
    psum = ctx.enter_context(tc.tile_pool(name="psum", bufs=4, space="PSUM"))
    psum_t = ctx.enter_context(tc.tile_pool(name="psum_t", bufs=2, space="PSUM"))
    psum_o = ctx.enter_context(tc.tile_pool(name="psum_o", bufs=1, space="PSUM"))

    # ---- W: loaded once per core into f32r (bit-identical copy);
    # emission deferred until after the first q-block's DMAs are queued
    wH = big.tile([P, EC * E], f32r, tag="wH")

    def emit_w_load():
        for ec in range(EC):
            win = qio.tile([P, E], f32, tag="qin", name="win")
            nc.scalar.dma_start(win[:], w_ap[ec * P:(ec + 1) * P, :])
            nc.scalar.copy(wH[:, ec * E:(ec + 1) * E], win[:])

    # deferred PV state: (b, row0, pt3, rz)
    pending = []

    def emit_pv(st, knat):
        b, row0, pt3, rz = st
        po = psum_o.tile([P, 2 * KB], f32, tag="po", name="po")
        for kc in range(KC):
            for eh in range(2):
                nc.tensor.matmul(
                    po[:, eh * KB:(eh + 1) * KB], pt3[:, kc, :],
                    knat[:, kc * E + eh * KB: kc * E + (eh + 1) * KB],
                    start=(kc == 0), stop=(kc == KC - 1))
        ot = oio.tile([P, 2 * KB], f32, tag="ot", name="ot")
        nc.scalar.activation(ot[:], po[:], AF.Copy, scale=rz[:])
        nc.gpsimd.dma_start(out_ap[b, row0: row0 + P, :], ot[:])

    def emit_block_qT(b, qb):
        q0 = qb * QB
        qTh = big.tile([P, EC, QB], f32r, tag="qTh", name="qTh")
        for qt in range(qt_per_b):
            qin = qio.tile([P, E], f32, tag="qin", name="qin")
            nc.sync.dma_start(
                qin[:], q_ap[b, q0 + qt * P: q0 + (qt + 1) * P, :])
            for eg in range(EC // GW):
                pt = psum_t.tile([P, GW * P], f32, tag="tp", name="pt")
                for j in range(GW):
                    ec = eg * GW + j
                    nc.tensor.transpose(
                        pt[:, j * P:(j + 1) * P],
                        qin[:, ec * P:(ec + 1) * P], ident[:])
                ptv = pt[:].rearrange("p (g c) -> p g c", g=GW)
                nc.scalar.copy(
                    qTh[:, eg * GW:(eg + 1) * GW, qt * P:(qt + 1) * P], ptv)
        return qTh

    def emit_block_qw(qTh):
        qwTh = big.tile([P, FC * QB], f32r, tag="qwTh", name="qwTh")
        for fg in range(FC // 2):
            ps = psum.tile([P, 2 * QB], f32, tag="ps", name="ps")
            for fi in range(2):
                fc = fg * 2 + fi
                for ec in range(EC):
                    nc.tensor.matmul(
                        ps[:, fi * QB:(fi + 1) * QB],
                        wH[:, ec * E + fc * P: ec * E + (fc + 1) * P],
                        qTh[:, ec, :],
                        start=(ec == 0), stop=(ec == EC - 1))
            nc.scalar.copy(qwTh[:, fg * 2 * QB:(fg + 1) * 2 * QB], ps[:])
        return qwTh

    def emit_k_phase(b):
        knat = big.tile([P, KC * E], bf16, tag="knat", name="knat")
        kTh = big.tile([P, EC, KL], f32r, tag="kTh", name="kTh")
        for kc in range(KC):
            kin = qio.tile([P, E], f32, tag="qin", name="kin")
            nc.sync.dma_start(kin[:], k_ap[b, kc * P:(kc + 1) * P, :])
            # bf16 copy for the PV matmul rhs
            nc.vector.tensor_copy(knat[:, kc * E:(kc + 1) * E], kin[:])
            for eg in range(EC // GW):
                pt = psum_t.tile([P, GW * P], f32, tag="tp", name="pt")
                for j in range(GW):
                    ec = eg * GW + j
                    nc.tensor.transpose(
                        pt[:, j * P:(j + 1) * P],
                        kin[:, ec * P:(ec + 1) * P], ident[:])
                ptv = pt[:].rearrange("p (g c) -> p g c", g=GW)
                nc.scalar.copy(
                    kTh[:, eg * GW:(eg + 1) * GW, kc * P:(kc + 1) * P], ptv)
        return knat, kTh

    for b in range(Bl):
        # first q-block prep runs before the K phase so the PE has work
        # while the k DMA stream lands
        qTh = emit_block_qT(b, 0)
        if b == 0:
            emit_w_load()
        qwTh = emit_block_qw(qTh)
        # flush deferred PVs of the previous batch before knat is rewritten
        while pending:
            emit_pv(pending.pop(0), knat)
        knat, kTh = emit_k_phase(b)

        qTh_next = None
        for qb in range(nqb):
            q0 = qb * QB
            if qb > 0:
                # qTh for this block was transposed mid-previous-block;
                # only the qw matmuls run here (their psum use is
                # sequential with this block's score matmuls)
                qwTh = emit_block_qw(qTh_next)

            for qt in range(qt_per_b):
                if qt == 1 and qb + 1 < nqb:
                    # transpose the next block's q mid-block so its ACT
                    # psum evacuations finish under this block's
                    # remaining score/PV matmuls
                    qTh_next = emit_block_qT(b, qb + 1)
                sp = spp.tile([P, KL], f32, tag="sp", name="sp")
                m4 = small.tile([P, nkb], f32, tag="m4", name="m4")
                for kb in range(nkb):
                    ps_s = psum.tile([P, KB], f32, tag="ps", name="ps_s")
                    for fc in range(FC):
                        qs = fc * QB + qt * P
                        nc.tensor.matmul(ps_s[:], qwTh[:, qs:qs + P],
                                         kTh[:, fc, kb * KB:(kb + 1) * KB],
                                         start=(fc == 0),
                                         stop=(fc == FC - 1))
                    nc.vector.tensor_copy(sp[:, kb * KB:(kb + 1) * KB],
                                          ps_s[:])
                    nc.vector.tensor_reduce(m4[:, kb:kb + 1], ps_s[:],
                                            axis=AX.X, op=ALU.max)

                negm = small.tile([P, 1], f32, tag="negm", name="negm")
                nc.vector.tensor_reduce(negm[:], m4[:], axis=AX.X,
                                        op=ALU.max, negate=True)
                z = small.tile([P, 1], f32, tag="z", name="z")
                e16 = e16p.tile([P, KL], bf16, tag="e16", name="e16")
                nc.scalar.activation(e16[:], sp[:], AF.Exp,
                                     bias=negm[:], accum_out=z[:])
                rz = small.tile([P, 1], f32, tag="rz", name="rz")
                nc.vector.reciprocal(rz[:], z[:])

                # multiplicative mask (applied after softmax numerator),
                # in place on the bf16 exp tile
                for kb in range(nkb):
                    mt = mio.tile([P, KB], i32, tag="mask", name="mt")
                    # gpsimd (SWDGE) dispatch: keeps the mask stream off
                    # the ACT queue so slot-waits can never delay exp
                    nc.gpsimd.dma_start(
                        mt[:], mask_ap[b, q0 + qt * P: q0 + (qt + 1) * P,
                                       kb * KB:(kb + 1) * KB])
                    nc.vector.scalar_tensor_tensor(
                        out=e16[:, kb * KB:(kb + 1) * KB], in0=mt[:],
                        scalar=1.0, in1=e16[:, kb * KB:(kb + 1) * KB],
                        op0=ALU.mult, op1=ALU.mult)

                # one XBAR transpose: p [128q, KL] -> pT chunks [128l, KC, 128q]
                pt3 = ptp.tile([P, KC, P], bf16, tag="pt3", name="pt3")
                nc.sync.dma_start(pt3[:], e16[:], transpose=True)

                pending.append((b, q0 + qt * P, pt3, rz))
                if len(pending) > 2:
                    emit_pv(pending.pop(0), knat)

    while pending:
        emit_pv(pending.pop(0), knat)


def build_program(Bl, Q, KL, E, QB=256):
    nc = bacc.Bacc("TRN2", target_bir_lowering=False, debug=False)
    k_t = nc.dram_tensor("k", [Bl, KL, E], dt.float32, kind="ExternalInput")
    q_t = nc.dram_tensor("q", [Bl, Q, E], dt.float32, kind="ExternalInput")
    w_t = nc.dram_tensor("W", [E, E], dt.float32, kind="ExternalInput")
    m_t = nc.dram_tensor("mask", [Bl, Q, KL], dt.int32, kind="ExternalInput")
    o_t = nc.dram_tensor("out", [Bl, Q, E], dt.float32, kind="ExternalOutput")
    with tile.TileContext(nc) as tc:
        with ExitStack() as ctx:
            emit_attention(ctx, tc, k_t.ap(), q_t.ap(), w_t.ap(), m_t.ap(),
                           o_t.ap(), Bl, Q, KL, E, QB=QB)
    nc.compile()
    return nc


def kernel(k: np.ndarray, q: np.ndarray, W: np.ndarray, mask: np.ndarray,
           **run_kwargs) -> np.ndarray:
    assert k.shape == (B, K_LEN, EMB) and q.shape == (B, Q_LEN, EMB)
    assert W.shape == (EMB, EMB) and mask.shape == (B, Q_LEN, K_LEN)
    Bl = B // N_CORES
    nc = build_program(Bl, Q_LEN, K_LEN, EMB)
    in_maps = []
    for c in range(N_CORES):
        sl = slice(c * Bl, (c + 1) * Bl)
        in_maps.append({
            "k": np.ascontiguousarray(k[sl], dtype=np.float32),
            "q": np.ascontiguousarray(q[sl], dtype=np.float32),
            "W": np.ascontiguousarray(W, dtype=np.float32),
            "mask": np.ascontiguousarray(mask[sl], dtype=np.int32),
        })
    res = run_bass_kernel_spmd(nc, in_maps, core_ids=list(range(N_CORES)),
                               **run_kwargs)
    out = np.concatenate([r["out"] for r in res.results], axis=0)
    if run_kwargs.get("trace"):
        kernel.last_exec_time_ns = res.exec_time_ns
    return out


kernel.last_exec_time_ns = None
